# revision 5
# baseline (speedup 1.0000x reference)
"""Trainium2 Bass kernel for NNBlendFM: 3-layer tanh MLP embedder + 64-head
rank-16 factorization machine, data-parallel over batch across 8 NeuronCores.

Math (per batch row b, head h):
    h = tanh(tanh(tanh(x W1 + b1) W2 + b2) W3 + b3)          # [B, 2048]
    lin[b,h]  = h . fm_w[h]
    vx[b,h,r] = h . fm_V[h,r]
    diag[b,h] = (h*h) . (sum_r fm_V[h,r]^2)
    out[h,b]  = fm_w0[h] + lin + 0.5*(sum_r vx^2 - diag)

Device layout: activations kept as [feature_partition, batch_free] tiles so
every matmul contracts over the partition dim with natural-layout weights as
the stationary operand.  The FM stage flips to [batch_partition, col_free] by
using h^T k-tiles as the stationary operand.  All matmul inputs are bf16
(fp32 PSUM accumulation), everything else fp32.
"""

import numpy as np
import ml_dtypes

import concourse.bass as bass
import concourse.tile as tile
from concourse import bacc, mybir
from concourse import bass_utils

BF16 = mybir.dt.bfloat16
F32 = mybir.dt.float32
AF = mybir.ActivationFunctionType
ALU = mybir.AluOpType

P = 128
IN, HID, HEADS, RANK = 512, 2048, 64, 16
B = 8192
NCORES = 8
BC = B // NCORES            # 1024 batch rows per core
KT1 = IN // P               # 4  k-tiles, layer 1
KT = HID // P               # 16 k-tiles, layers 2/3 + FM
JT = HID // P               # 16 output-feature tiles per layer
NB = 512                    # matmul moving free-dim (one PSUM bank)
NBC = BC // NB              # 2 batch column chunks
BT = BC // P                # 8 batch tiles in FM stage
HR = HEADS * RANK           # 1024 vx columns

_CACHE = {}


def _build_module():
    nc = bacc.Bacc(
        "TRN2", target_bir_lowering=False, debug=False, num_devices=NCORES
    )
    dt = nc.dram_tensor
    xT = dt("xT", [IN, BC], BF16, kind="ExternalInput").ap()
    W1 = dt("W1", [IN, HID], BF16, kind="ExternalInput").ap()
    W2 = dt("W2", [HID, HID], BF16, kind="ExternalInput").ap()
    W3 = dt("W3", [HID, HID], BF16, kind="ExternalInput").ap()
    B1 = dt("B1", [P, JT], F32, kind="ExternalInput").ap()
    B2 = dt("B2", [P, JT], F32, kind="ExternalInput").ap()
    B3 = dt("B3", [P, JT], F32, kind="ExternalInput").ap()
    VT = dt("VT", [HID, HR], BF16, kind="ExternalInput").ap()
    FW = dt("FW", [P, KT * HEADS], BF16, kind="ExternalInput").ap()
    SQ = dt("SQ", [P, KT * HEADS], BF16, kind="ExternalInput").ap()
    W0 = dt("W0", [P, HEADS], F32, kind="ExternalInput").ap()
    OUT = dt("out", [BC, HEADS], F32, kind="ExternalOutput").ap()

    with tile.TileContext(nc) as tc:
        with (
            tc.tile_pool(name="wpool", bufs=24) as wpool,
            tc.tile_pool(name="hpool", bufs=32) as hpool,
            tc.tile_pool(name="vtpool", bufs=16) as vtpool,
            tc.tile_pool(name="cpool", bufs=1) as cpool,
            tc.tile_pool(name="pp", bufs=8, space="PSUM") as pp,
            tc.tile_pool(name="epool", bufs=2) as epool,
            tc.tile_pool(name="spool", bufs=8) as spool,
            tc.tile_pool(name="opool", bufs=4) as opool,
        ):
            # --- DMA issue order is the program order below.  The Sync
            # sequencer issues one dma_start per ~0.6us, so the critical
            # first-matmul inputs (x, W1) go first, then per-layer weights in
            # use order; FM-stage operands (VT/FW/SQ) last — W3's WAR-stalled
            # issues naturally delay them to mid-kernel.
            def load_w(dram, ktiles, name):
                ts = []
                for k in range(ktiles):
                    w_k = wpool.tile([P, HID], BF16, tag="w", name=f"{name}_{k}")
                    nc.sync.dma_start(w_k[:], dram[k * P : (k + 1) * P, :])
                    ts.append(w_k)
                return ts

            # x^T on GpSimd's DMA queue, W1 on Sync's — both sequencers issue
            # in parallel so layer-1's inputs land ~2x sooner.
            xt = []
            w1t = []
            for k in range(KT1):
                x_k = hpool.tile([P, BC], BF16, tag="h", name=f"xt{k}")
                nc.gpsimd.dma_start(x_k[:], xT[k * P : (k + 1) * P, :])
                xt.append(x_k)
                w_k = wpool.tile([P, HID], BF16, tag="w", name=f"w1_{k}")
                nc.sync.dma_start(w_k[:], W1[k * P : (k + 1) * P, :])
                w1t.append(w_k)

            b1t = cpool.tile([P, JT], F32, tag="b1")
            nc.sync.dma_start(b1t[:], B1)
            b2t = cpool.tile([P, JT], F32, tag="b2")
            nc.sync.dma_start(b2t[:], B2)
            b3t = cpool.tile([P, JT], F32, tag="b3")
            nc.sync.dma_start(b3t[:], B3)
            w0t = cpool.tile([P, HEADS], F32, tag="w0")
            nc.sync.dma_start(w0t[:], W0)

            w2t = load_w(W2, KT, "w2")
            w3t = load_w(W3, KT, "w3")

            # FM operands: issued after W3 so they don't crowd the head.
            vtt = []
            for k in range(KT):
                vt_k = vtpool.tile([P, HR], BF16, tag="vt", name=f"vt{k}")
                nc.sync.dma_start(vt_k[:], VT[k * P : (k + 1) * P, :])
                vtt.append(vt_k)
            fwt = cpool.tile([P, KT * HEADS], BF16, tag="fw")
            nc.sync.dma_start(fwt[:], FW)
            sqt = cpool.tile([P, KT * HEADS], BF16, tag="sq")
            nc.sync.dma_start(sqt[:], SQ)

            # --- embedder layers ------------------------------------------
            def layer(h_prev, w_tiles, bias_t, ktiles, name):
                h_out = []
                for jt in range(JT):
                    ps = []
                    for c in range(NBC):
                        ps_c = pp.tile([P, NB], F32, tag="ps", name=f"{name}ps{jt}_{c}")
                        ps.append(ps_c)
                    for kt in range(ktiles):
                        lhsT = w_tiles[kt][:, jt * P : (jt + 1) * P]
                        for c in range(NBC):
                            nc.tensor.matmul(
                                ps[c][:],
                                lhsT,
                                h_prev[kt][:, c * NB : (c + 1) * NB],
                                start=(kt == 0),
                                stop=(kt == ktiles - 1),
                            )
                    ht = hpool.tile([P, BC], BF16, tag="h", name=f"{name}h{jt}")
                    for c in range(NBC):
                        nc.scalar.activation(
                            ht[:, c * NB : (c + 1) * NB],
                            ps[c][:],
                            AF.Tanh,
                            bias=bias_t[:, jt : jt + 1],
                        )
                    h_out.append(ht)
                return h_out

            h1 = layer(xt, w1t, b1t, KT1, "l1")
            h2 = layer(h1, w2t, b2t, KT, "l2")
            h3 = layer(h2, w3t, b3t, KT, "l3")

            # --- h3 squared (stationary operand for the diag matmuls) -----
            h3sq = []
            for k in range(KT):
                sq_k = hpool.tile([P, BC], BF16, tag="h", name=f"h3sq{k}")
                nc.vector.tensor_mul(sq_k[:], h3[k][:], h3[k][:])
                h3sq.append(sq_k)

            # --- FM stage: per 128-row batch tile -------------------------
            def fm_phase_a(bt):
                """vx = h V^T (1024 cols) and lin = h fm_w^T (64 cols)."""
                vx0 = pp.tile([P, NB], F32, tag="ps", name=f"vx0_{bt}")
                vx1 = pp.tile([P, NB], F32, tag="ps", name=f"vx1_{bt}")
                lw = pp.tile([P, NB], F32, tag="ps", name=f"lw_{bt}")
                bsl = slice(bt * P, (bt + 1) * P)
                for kt in range(KT):
                    lhsT = h3[kt][:, bsl]
                    nc.tensor.matmul(
                        vx0[:], lhsT, vtt[kt][:, 0:NB],
                        start=(kt == 0), stop=(kt == KT - 1),
                    )
                    nc.tensor.matmul(
                        vx1[:], lhsT, vtt[kt][:, NB:HR],
                        start=(kt == 0), stop=(kt == KT - 1),
                    )
                    nc.tensor.matmul(
                        lw[:, 0:HEADS], lhsT,
                        fwt[:, kt * HEADS : (kt + 1) * HEADS],
                        start=(kt == 0), stop=(kt == KT - 1),
                    )
                return vx0, vx1, lw

            def fm_phase_b(bt):
                """diag = (h*h) . (0.5 * sum_r V^2), already scaled by 0.5."""
                dg = pp.tile([P, NB], F32, tag="ps", name=f"dg_{bt}")
                bsl = slice(bt * P, (bt + 1) * P)
                for kt in range(KT):
                    nc.tensor.matmul(
                        dg[:, 0:HEADS],
                        h3sq[kt][:, bsl],
                        sqt[:, kt * HEADS : (kt + 1) * HEADS],
                        start=(kt == 0), stop=(kt == KT - 1),
                    )
                return dg

            def fm_epilogue(bt, vx0, vx1, lw, dg):
                vx2 = epool.tile([P, HR], F32, tag="e", name=f"vx2_{bt}")
                nc.scalar.activation(vx2[:, 0:NB], vx0[:], AF.Square)
                nc.scalar.activation(vx2[:, NB:HR], vx1[:], AF.Square)
                sumv = spool.tile([P, HEADS], F32, tag="s", name=f"sumv_{bt}")
                nc.vector.reduce_sum(
                    sumv[:],
                    vx2[:].rearrange("p (h r) -> p h r", r=RANK),
                    axis=mybir.AxisListType.X,
                )
                # q = 0.5*sumv - diag_half
                q = spool.tile([P, HEADS], F32, tag="s", name=f"q_{bt}")
                nc.vector.scalar_tensor_tensor(
                    q[:], sumv[:], 0.5, dg[:, 0:HEADS],
                    op0=ALU.mult, op1=ALU.subtract,
                )
                t = spool.tile([P, HEADS], F32, tag="s", name=f"t_{bt}")
                nc.vector.tensor_add(t[:], q[:], lw[:, 0:HEADS])
                ot = opool.tile([P, HEADS], F32, tag="o", name=f"ot_{bt}")
                nc.vector.tensor_add(ot[:], t[:], w0t[:])
                nc.sync.dma_start(OUT[bt * P : (bt + 1) * P, :], ot[:])

            # Stagger: A(0), A(1), B(0), E(0), A(2), B(1), E(1), ...
            pend = []  # (bt, vx0, vx1, lw)
            for bt in range(BT):
                pend.append((bt, *fm_phase_a(bt)))
                if len(pend) == 2:
                    obt, vx0, vx1, lw = pend.pop(0)
                    dg = fm_phase_b(obt)
                    fm_epilogue(obt, vx0, vx1, lw, dg)
            while pend:
                obt, vx0, vx1, lw = pend.pop(0)
                dg = fm_phase_b(obt)
                fm_epilogue(obt, vx0, vx1, lw, dg)

    nc.compile()
    return nc


def _get_nc():
    if "nc" not in _CACHE:
        _CACHE["nc"] = _build_module()
    return _CACHE["nc"]


def _prep_host(x, W1, b1, W2, b2, W3, b3, fm_w0, fm_w, fm_V):
    """Host-side layout prep: bf16 casts, transposes, per-head V reductions."""
    bf = ml_dtypes.bfloat16
    f32 = np.float32

    common = {
        "W1": np.ascontiguousarray(W1.astype(bf)),
        "W2": np.ascontiguousarray(W2.astype(bf)),
        "W3": np.ascontiguousarray(W3.astype(bf)),
        "B1": np.ascontiguousarray(b1.astype(f32).reshape(JT, P).T),
        "B2": np.ascontiguousarray(b2.astype(f32).reshape(JT, P).T),
        "B3": np.ascontiguousarray(b3.astype(f32).reshape(JT, P).T),
        # V^T: [2048, heads*rank], col hr = h*RANK + r
        "VT": np.ascontiguousarray(
            fm_V.reshape(HEADS * RANK, HID).T.astype(bf)
        ),
        # fm_w^T packed as [128, kt*64]: FW[p, kt*64+h] = fm_w[h, kt*128+p]
        "FW": np.ascontiguousarray(
            fm_w.T.reshape(KT, P, HEADS).transpose(1, 0, 2).reshape(P, KT * HEADS)
            .astype(bf)
        ),
        # 0.5 * sum_r V^2, same packing
        "SQ": np.ascontiguousarray(
            (0.5 * (fm_V.astype(np.float64) ** 2).sum(axis=1))
            .T.reshape(KT, P, HEADS).transpose(1, 0, 2).reshape(P, KT * HEADS)
            .astype(bf)
        ),
        "W0": np.ascontiguousarray(
            np.tile(fm_w0.astype(f32)[None, :], (P, 1))
        ),
    }

    in_maps = []
    xb = x.astype(bf)
    for c in range(NCORES):
        m = dict(common)
        m["xT"] = np.ascontiguousarray(xb[c * BC : (c + 1) * BC, :].T)
        in_maps.append(m)
    return in_maps


def kernel(x, W1, b1, W2, b2, W3, b3, fm_w0, fm_w, fm_V):
    nc = _get_nc()
    in_maps = _prep_host(x, W1, b1, W2, b2, W3, b3, fm_w0, fm_w, fm_V)
    import os
    trace = bool(int(os.environ.get("KERNEL_TRACE", "0")))
    res = bass_utils.run_bass_kernel_spmd(
        nc, in_maps, core_ids=list(range(NCORES)), trace=trace,
    )
    _CACHE["last_results"] = res
    outs = [res.results[c]["out"] for c in range(NCORES)]
    full = np.concatenate(outs, axis=0)          # [B, HEADS]
    return np.ascontiguousarray(full.T).astype(np.float32)  # [HEADS, B]


# revision 8
# speedup vs baseline: 1.0048x; 1.0048x over previous
"""Trainium2 Bass kernel for NNBlendFM: 3-layer tanh MLP embedder + 64-head
rank-16 factorization machine, data-parallel over batch across 8 NeuronCores.

Math (per batch row b, head h):
    h = tanh(tanh(tanh(x W1 + b1) W2 + b2) W3 + b3)          # [B, 2048]
    lin[b,h]  = h . fm_w[h]
    vx[b,h,r] = h . fm_V[h,r]
    diag[b,h] = (h*h) . (sum_r fm_V[h,r]^2)
    out[h,b]  = fm_w0[h] + lin + 0.5*(sum_r vx^2 - diag)

Device layout: activations kept as [feature_partition, batch_free] tiles so
every matmul contracts over the partition dim with natural-layout weights as
the stationary operand.  The FM stage flips to [batch_partition, col_free] by
using h^T k-tiles as the stationary operand.  All matmul inputs are bf16
(fp32 PSUM accumulation), everything else fp32.
"""

import numpy as np
import ml_dtypes

import concourse.bass as bass
import concourse.tile as tile
from concourse import bacc, mybir
from concourse import bass_utils

BF16 = mybir.dt.bfloat16
F32 = mybir.dt.float32
AF = mybir.ActivationFunctionType
ALU = mybir.AluOpType

P = 128
IN, HID, HEADS, RANK = 512, 2048, 64, 16
B = 8192
NCORES = 8
BC = B // NCORES            # 1024 batch rows per core
KT1 = IN // P               # 4  k-tiles, layer 1
KT = HID // P               # 16 k-tiles, layers 2/3 + FM
JT = HID // P               # 16 output-feature tiles per layer
NB = 512                    # matmul moving free-dim (one PSUM bank)
NBC = BC // NB              # 2 batch column chunks
BT = BC // P                # 8 batch tiles in FM stage
HR = HEADS * RANK           # 1024 vx columns

_CACHE = {}


def _build_module():
    nc = bacc.Bacc(
        "TRN2", target_bir_lowering=False, debug=False, num_devices=NCORES
    )
    dt = nc.dram_tensor
    xT = dt("xT", [IN, BC], BF16, kind="ExternalInput").ap()
    W1 = dt("W1", [IN, HID], BF16, kind="ExternalInput").ap()
    W2 = dt("W2", [HID, HID], BF16, kind="ExternalInput").ap()
    W3 = dt("W3", [HID, HID], BF16, kind="ExternalInput").ap()
    B1 = dt("B1", [P, JT], F32, kind="ExternalInput").ap()
    B2 = dt("B2", [P, JT], F32, kind="ExternalInput").ap()
    B3 = dt("B3", [P, JT], F32, kind="ExternalInput").ap()
    VT = dt("VT", [HID, HR], BF16, kind="ExternalInput").ap()
    FW = dt("FW", [P, KT * HEADS], BF16, kind="ExternalInput").ap()
    SQ = dt("SQ", [P, KT * HEADS], BF16, kind="ExternalInput").ap()
    W0 = dt("W0", [P, HEADS], F32, kind="ExternalInput").ap()
    OUT = dt("out", [BC, HEADS], F32, kind="ExternalOutput").ap()

    with tile.TileContext(nc) as tc:
        with (
            tc.tile_pool(name="wpool", bufs=24) as wpool,
            tc.tile_pool(name="hpool", bufs=32) as hpool,
            tc.tile_pool(name="vtpool", bufs=16) as vtpool,
            tc.tile_pool(name="cpool", bufs=1) as cpool,
            tc.tile_pool(name="pp", bufs=8, space="PSUM") as pp,
            tc.tile_pool(name="epool", bufs=2) as epool,
            tc.tile_pool(name="spool", bufs=8) as spool,
            tc.tile_pool(name="opool", bufs=4) as opool,
        ):
            # --- DMA issue order is the program order below.  The Sync
            # sequencer issues one dma_start per ~0.6us, so the critical
            # first-matmul inputs (x, W1) go first, then per-layer weights in
            # use order; FM-stage operands (VT/FW/SQ) last — W3's WAR-stalled
            # issues naturally delay them to mid-kernel.
            def load_w(dram, ktiles, name):
                ts = []
                for k in range(ktiles):
                    w_k = wpool.tile([P, HID], BF16, tag="w", name=f"{name}_{k}")
                    nc.sync.dma_start(w_k[:], dram[k * P : (k + 1) * P, :])
                    ts.append(w_k)
                return ts

            # Fan the critical layer-1 inputs across four otherwise-idle
            # engine sequencers so their dma_start issues run in parallel
            # (each issue occupies a sequencer ~0.6us).
            eng = [nc.gpsimd, nc.scalar, nc.gpsimd, nc.scalar]
            xt = []
            w1t = []
            for k in range(KT1):
                x_k = hpool.tile([P, BC], BF16, tag="h", name=f"xt{k}")
                eng[k].dma_start(x_k[:], xT[k * P : (k + 1) * P, :])
                xt.append(x_k)
                w_k = wpool.tile([P, HID], BF16, tag="w", name=f"w1_{k}")
                nc.sync.dma_start(w_k[:], W1[k * P : (k + 1) * P, :])
                w1t.append(w_k)

            b1t = cpool.tile([P, JT], F32, tag="b1")
            nc.gpsimd.dma_start(b1t[:], B1)
            b2t = cpool.tile([P, JT], F32, tag="b2")
            nc.gpsimd.dma_start(b2t[:], B2)
            b3t = cpool.tile([P, JT], F32, tag="b3")
            nc.gpsimd.dma_start(b3t[:], B3)
            w0t = cpool.tile([P, HEADS], F32, tag="w0")
            nc.gpsimd.dma_start(w0t[:], W0)

            w2t = load_w(W2, KT, "w2")
            w3t = load_w(W3, KT, "w3")

            # FM operands: issued after W3 so they don't crowd the head.
            vtt = []
            for k in range(KT):
                vt_k = vtpool.tile([P, HR], BF16, tag="vt", name=f"vt{k}")
                nc.sync.dma_start(vt_k[:], VT[k * P : (k + 1) * P, :])
                vtt.append(vt_k)
            fwt = cpool.tile([P, KT * HEADS], BF16, tag="fw")
            nc.sync.dma_start(fwt[:], FW)
            sqt = cpool.tile([P, KT * HEADS], BF16, tag="sq")
            nc.sync.dma_start(sqt[:], SQ)

            # --- embedder layers ------------------------------------------
            def layer(h_prev, w_tiles, bias_t, ktiles, name):
                h_out = []
                for jt in range(JT):
                    ps = []
                    for c in range(NBC):
                        ps_c = pp.tile([P, NB], F32, tag="ps", name=f"{name}ps{jt}_{c}")
                        ps.append(ps_c)
                    # Rotate the accumulation order by jt so each weight
                    # tile's final read retires early for some jt, releasing
                    # its pool slot for the next layer's prefetch DMA.
                    kts = [(kt + jt) % ktiles for kt in range(ktiles)]
                    for i, kt in enumerate(kts):
                        lhsT = w_tiles[kt][:, jt * P : (jt + 1) * P]
                        for c in range(NBC):
                            nc.tensor.matmul(
                                ps[c][:],
                                lhsT,
                                h_prev[kt][:, c * NB : (c + 1) * NB],
                                start=(i == 0),
                                stop=(i == ktiles - 1),
                            )
                    ht = hpool.tile([P, BC], BF16, tag="h", name=f"{name}h{jt}")
                    for c in range(NBC):
                        nc.scalar.activation(
                            ht[:, c * NB : (c + 1) * NB],
                            ps[c][:],
                            AF.Tanh,
                            bias=bias_t[:, jt : jt + 1],
                        )
                    h_out.append(ht)
                return h_out

            h1 = layer(xt, w1t, b1t, KT1, "l1")
            h2 = layer(h1, w2t, b2t, KT, "l2")
            h3 = layer(h2, w3t, b3t, KT, "l3")

            # --- h3 squared (stationary operand for the diag matmuls) -----
            h3sq = []
            for k in range(KT):
                sq_k = hpool.tile([P, BC], BF16, tag="h", name=f"h3sq{k}")
                nc.vector.tensor_mul(sq_k[:], h3[k][:], h3[k][:])
                h3sq.append(sq_k)

            # --- FM stage: per 128-row batch tile -------------------------
            def fm_phase_a(bt):
                """vx = h V^T (1024 cols) and lin = h fm_w^T (64 cols)."""
                vx0 = pp.tile([P, NB], F32, tag="ps", name=f"vx0_{bt}")
                vx1 = pp.tile([P, NB], F32, tag="ps", name=f"vx1_{bt}")
                lw = pp.tile([P, NB], F32, tag="ps", name=f"lw_{bt}")
                bsl = slice(bt * P, (bt + 1) * P)
                for kt in range(KT):
                    lhsT = h3[kt][:, bsl]
                    nc.tensor.matmul(
                        vx0[:], lhsT, vtt[kt][:, 0:NB],
                        start=(kt == 0), stop=(kt == KT - 1),
                    )
                    nc.tensor.matmul(
                        vx1[:], lhsT, vtt[kt][:, NB:HR],
                        start=(kt == 0), stop=(kt == KT - 1),
                    )
                    nc.tensor.matmul(
                        lw[:, 0:HEADS], lhsT,
                        fwt[:, kt * HEADS : (kt + 1) * HEADS],
                        start=(kt == 0), stop=(kt == KT - 1),
                    )
                return vx0, vx1, lw

            def fm_phase_b(bt):
                """diag = (h*h) . (0.5 * sum_r V^2), already scaled by 0.5."""
                dg = pp.tile([P, NB], F32, tag="ps", name=f"dg_{bt}")
                bsl = slice(bt * P, (bt + 1) * P)
                for kt in range(KT):
                    nc.tensor.matmul(
                        dg[:, 0:HEADS],
                        h3sq[kt][:, bsl],
                        sqt[:, kt * HEADS : (kt + 1) * HEADS],
                        start=(kt == 0), stop=(kt == KT - 1),
                    )
                return dg

            def fm_epilogue(bt, vx0, vx1, lw, dg):
                vx2 = epool.tile([P, HR], F32, tag="e", name=f"vx2_{bt}")
                nc.scalar.activation(vx2[:, 0:NB], vx0[:], AF.Square)
                nc.scalar.activation(vx2[:, NB:HR], vx1[:], AF.Square)
                sumv = spool.tile([P, HEADS], F32, tag="s", name=f"sumv_{bt}")
                nc.vector.reduce_sum(
                    sumv[:],
                    vx2[:].rearrange("p (h r) -> p h r", r=RANK),
                    axis=mybir.AxisListType.X,
                )
                # q = 0.5*sumv - diag_half
                q = spool.tile([P, HEADS], F32, tag="s", name=f"q_{bt}")
                nc.vector.scalar_tensor_tensor(
                    q[:], sumv[:], 0.5, dg[:, 0:HEADS],
                    op0=ALU.mult, op1=ALU.subtract,
                )
                t = spool.tile([P, HEADS], F32, tag="s", name=f"t_{bt}")
                nc.vector.tensor_add(t[:], q[:], lw[:, 0:HEADS])
                ot = opool.tile([P, HEADS], F32, tag="o", name=f"ot_{bt}")
                nc.vector.tensor_add(ot[:], t[:], w0t[:])
                nc.sync.dma_start(OUT[bt * P : (bt + 1) * P, :], ot[:])

            # Stagger: A(0), A(1), B(0), E(0), A(2), B(1), E(1), ...
            pend = []  # (bt, vx0, vx1, lw)
            for bt in range(BT):
                pend.append((bt, *fm_phase_a(bt)))
                if len(pend) == 2:
                    obt, vx0, vx1, lw = pend.pop(0)
                    dg = fm_phase_b(obt)
                    fm_epilogue(obt, vx0, vx1, lw, dg)
            while pend:
                obt, vx0, vx1, lw = pend.pop(0)
                dg = fm_phase_b(obt)
                fm_epilogue(obt, vx0, vx1, lw, dg)

    nc.compile()
    return nc


def _get_nc():
    if "nc" not in _CACHE:
        _CACHE["nc"] = _build_module()
    return _CACHE["nc"]


def _prep_host(x, W1, b1, W2, b2, W3, b3, fm_w0, fm_w, fm_V):
    """Host-side layout prep: bf16 casts, transposes, per-head V reductions."""
    bf = ml_dtypes.bfloat16
    f32 = np.float32

    common = {
        "W1": np.ascontiguousarray(W1.astype(bf)),
        "W2": np.ascontiguousarray(W2.astype(bf)),
        "W3": np.ascontiguousarray(W3.astype(bf)),
        "B1": np.ascontiguousarray(b1.astype(f32).reshape(JT, P).T),
        "B2": np.ascontiguousarray(b2.astype(f32).reshape(JT, P).T),
        "B3": np.ascontiguousarray(b3.astype(f32).reshape(JT, P).T),
        # V^T: [2048, heads*rank], col hr = h*RANK + r
        "VT": np.ascontiguousarray(
            fm_V.reshape(HEADS * RANK, HID).T.astype(bf)
        ),
        # fm_w^T packed as [128, kt*64]: FW[p, kt*64+h] = fm_w[h, kt*128+p]
        "FW": np.ascontiguousarray(
            fm_w.T.reshape(KT, P, HEADS).transpose(1, 0, 2).reshape(P, KT * HEADS)
            .astype(bf)
        ),
        # 0.5 * sum_r V^2, same packing
        "SQ": np.ascontiguousarray(
            (0.5 * (fm_V.astype(np.float64) ** 2).sum(axis=1))
            .T.reshape(KT, P, HEADS).transpose(1, 0, 2).reshape(P, KT * HEADS)
            .astype(bf)
        ),
        "W0": np.ascontiguousarray(
            np.tile(fm_w0.astype(f32)[None, :], (P, 1))
        ),
    }

    in_maps = []
    xb = x.astype(bf)
    for c in range(NCORES):
        m = dict(common)
        m["xT"] = np.ascontiguousarray(xb[c * BC : (c + 1) * BC, :].T)
        in_maps.append(m)
    return in_maps


def kernel(x, W1, b1, W2, b2, W3, b3, fm_w0, fm_w, fm_V):
    nc = _get_nc()
    in_maps = _prep_host(x, W1, b1, W2, b2, W3, b3, fm_w0, fm_w, fm_V)
    import os
    trace = bool(int(os.environ.get("KERNEL_TRACE", "0")))
    res = bass_utils.run_bass_kernel_spmd(
        nc, in_maps, core_ids=list(range(NCORES)), trace=trace,
    )
    _CACHE["last_results"] = res
    outs = [res.results[c]["out"] for c in range(NCORES)]
    full = np.concatenate(outs, axis=0)          # [B, HEADS]
    return np.ascontiguousarray(full.T).astype(np.float32)  # [HEADS, B]
